# revision 16
# baseline (speedup 1.0000x reference)
"""ADDS loss kernel for Trainium2, SPMD over 8 NeuronCores.

Problem: pred = model_points @ pred_R^T + pred_t (per batch), gt likewise;
d2[b,n,m] = ||pred[b,n] - gt[b,m]||^2; out = mean_{b,n} sqrt(max(min_m d2, 0)).

Sharding: data-parallel over batch B=32 -> 4 batches per core, one 5-row
operand group per batch at partition base 32*b:
  pred_stuff rows = [-2*p_x, -2*p_y, -2*p_z, pn2, 1]
  gt_stuff   rows = [g_x, g_y, g_z, 1, gn2]
so a K=5 matmul yields d2[n, m] = -2 p.g + pn2[n] + gn2[m] directly in PSUM.

Reduction: per (n_chunk, batch) the 4 m-chunk PSUM tiles are consumed by
ACT (bf16 convert of 2 tiles to SBUF) + DVE (2 chained tensor_tensor_scan
running-mins over (PSUM tile, SBUF tile) pairs) -> [128,1] min with no
reduce tail. Then clamp, sqrt, sum. Host sums the 8x[128,1] partials.
"""

import numpy as np

import concourse.bass as bass
import concourse.bacc as bacc_mod
import concourse.mybir as mybir
from concourse.tile import TileContext
from concourse.bass_utils import run_bass_kernel_spmd

B = 32
N = 2048
NCORES = 8
BPC = B // NCORES  # batches per core = 4
FP32 = mybir.dt.float32
BF16 = mybir.dt.bfloat16
AF = mybir.ActivationFunctionType
OP = mybir.AluOpType

# float32r streams at 1 col/cycle for moving dims >= 256 (vs 1/4 for fp32)
USE_F32R = True
BIG_INIT = 1.0e30
NFILL = 3


def _mm_dt(ap):
    return ap.bitcast(mybir.dt.float32r) if USE_F32R else ap


def build_kernel():
    nc = bacc_mod.Bacc()

    pointsT_ext = nc.declare_dram_parameter("pointsT", [3, N], FP32, isOutput=False)
    Rp_ext = nc.declare_dram_parameter("Rp", [3, 128], FP32, isOutput=False)
    Rg_ext = nc.declare_dram_parameter("Rg", [3, 128], FP32, isOutput=False)
    biasp_ext = nc.declare_dram_parameter("biasp", [128, 1], FP32, isOutput=False)
    biasg_ext = nc.declare_dram_parameter("biasg", [128, 1], FP32, isOutput=False)
    onesp_ext = nc.declare_dram_parameter("onesp", [128, 128], FP32, isOutput=False)
    onesg_ext = nc.declare_dram_parameter("onesg", [128, 128], FP32, isOutput=False)
    out_ext = nc.declare_dram_parameter("out", [128, 1], FP32, isOutput=True)
    warm_ext = nc.declare_dram_parameter("warm", [128, 1], FP32, isOutput=True)

    with TileContext(nc) as tc:
        with (
            tc.tile_pool(name="persist", bufs=1) as persist,
            tc.tile_pool(name="work", bufs=2) as work,
            tc.tile_pool(name="sbf", bufs=3) as sbf,
            tc.tile_pool(name="ps", bufs=3, space="PSUM") as ps,
        ):
            # ---- load inputs ----
            def load(ext, shape, nm):
                t = persist.tile(shape, FP32, tag=nm, name=nm)
                nc.sync.dma_start(out=t[:, :], in_=ext[:, :])
                return t

            pointsT = load(pointsT_ext, [3, N], "pointsT_sb")
            Rsb = {}
            biassb = {}
            onessb = {}
            for side, (R_ext, b_ext, o_ext) in (
                ("p", (Rp_ext, biasp_ext, onesp_ext)),
                ("g", (Rg_ext, biasg_ext, onesg_ext)),
            ):
                Rsb[side] = load(R_ext, [3, 128], f"R{side}_sb")
                biassb[side] = load(b_ext, [128, 1], f"bias{side}_sb")
                onessb[side] = load(o_ext, [128, 128], f"ones{side}_sb")

            # ---- Phase A: build stuff_p / stuff_g (fp32, then round to f32r) ----
            stuff = {}
            for side, scale in (("p", -2.0), ("g", 1.0)):
                stp = persist.tile([128, N], FP32, tag=f"stp{side}", name=f"stp{side}_sb")
                for c in range(N // 512):
                    T = ps.tile([128, 512], FP32, tag="psb", name="psb")
                    nc.tensor.matmul(
                        T[:, :],
                        Rsb[side][:, :],
                        pointsT[:, c * 512 : (c + 1) * 512],
                        start=True,
                        stop=True,
                    )
                    # stuff = scale*transform + bias
                    nc.scalar.activation(
                        stp[:, c * 512 : (c + 1) * 512], T[:, :], AF.Identity,
                        bias=biassb[side][:, :], scale=scale,
                    )
                # square on ACT
                sq = work.tile([128, N], FP32, tag="sq", name="sq")
                nc.scalar.activation(sq[:, :], stp[:, :], AF.Square)
                # norms matmul: N_ps[row, n] = sum_p ones[p, row] * sq[p, n]
                for c in range(N // 512):
                    N_ps = ps.tile([128, 512], FP32, tag="psb", name="psb")
                    nc.tensor.matmul(
                        N_ps[:, :],
                        onessb[side][:, :],
                        sq[:, c * 512 : (c + 1) * 512],
                        start=True,
                        stop=True,
                    )
                    # fold norm rows into stuff (other rows of N_ps are 0)
                    nc.vector.tensor_tensor(
                        stp[:, c * 512 : (c + 1) * 512],
                        stp[:, c * 512 : (c + 1) * 512],
                        N_ps[:, :],
                        op=OP.add,
                    )
                if USE_F32R:
                    st = persist.tile(
                        [128, N], mybir.dt.float32r,
                        tag=f"stuff{side}", name=f"stuff{side}_sb",
                    )
                    nc.scalar.copy(st[:, :], stp[:, :])
                    stuff[side] = st
                else:
                    stuff[side] = stp

            # ---- Phase B: main loop ----
            # Per (nch, b) group: two [128,1024] PSUM tiles (2 banks each,
            # 2 f32r matmuls per tile). Groups alternate consumers:
            #  - DVE-direct: reduce_min over each 1024-wide tile.
            #  - ACT+tree: ACT converts each tile half to bf16 SBUF, DVE runs
            #    a 2x TT-min tree (1024->512->256->128) + split final reduce.
            # Every group emits 2 partial mins into mins2[:, b, 0:2].
            roots = persist.tile([128, 16 * BPC], FP32, tag="roots", name="roots")
            for nch in range(16):
                mins2 = work.tile([128, BPC, 2], FP32, tag="mins2", name="mins2")
                for b in range(BPC):
                    g = nch * BPC + b
                    lhs = stuff["p"][32 * b : 32 * b + 5, nch * 128 : (nch + 1) * 128]
                    halves = []
                    for h in range(2):
                        P = ps.tile([128, 1024], FP32, tag="psb", name="psb")
                        halves.append(P)
                        for mc in range(2):
                            m0 = (2 * h + mc) * 512
                            nc.tensor.matmul(
                                P[:, mc * 512 : (mc + 1) * 512],
                                lhs,
                                stuff["g"][32 * b : 32 * b + 5, m0 : m0 + 512],
                                start=True,
                                stop=True,
                                tile_position=(32 * b, 0),
                            )
                    # HAM-warming fillers: keep the PE near-continuously
                    # busy so it runs at 2.4 GHz; results are discarded.
                    warmps = ps.tile(
                        [128, 512], FP32, tag="warm", name="warmps", bufs=1
                    )
                    for _f in range(NFILL):
                        nc.tensor.matmul(
                            warmps[:, :],
                            stuff["p"][0:1, 0:128],
                            stuff["g"][0:1, 0:512],
                            start=True,
                            stop=True,
                        )
                    if g % 7 == 6:
                        # DVE-direct group
                        for h in range(2):
                            nc.vector.tensor_reduce(
                                mins2[:, b, h : h + 1], halves[h][:, :],
                                axis=mybir.AxisListType.X, op=OP.min,
                            )
                    else:
                        S = sbf.tile([128, 2048], BF16, tag="S", name="S")
                        for h in range(2):
                            nc.scalar.copy(
                                S[:, h * 1024 : (h + 1) * 1024], halves[h][:, :]
                            )
                        u1 = sbf.tile([128, 1024], BF16, tag="u1", name="u1")
                        nc.vector.tensor_tensor(
                            u1[:, :], S[:, 0:1024], S[:, 1024:2048], op=OP.min
                        )
                        u2 = sbf.tile([128, 512], BF16, tag="u2", name="u2")
                        nc.vector.tensor_tensor(
                            u2[:, :], u1[:, 0:512], u1[:, 512:1024], op=OP.min
                        )
                        u3 = sbf.tile([128, 256], BF16, tag="u3", name="u3")
                        nc.vector.tensor_tensor(
                            u3[:, :], u2[:, 0:256], u2[:, 256:512], op=OP.min
                        )
                        u4 = sbf.tile([128, 128], BF16, tag="u4", name="u4")
                        nc.vector.tensor_tensor(
                            u4[:, :], u3[:, 0:128], u3[:, 128:256], op=OP.min
                        )
                        for h in range(2):
                            nc.vector.tensor_reduce(
                                mins2[:, b, h : h + 1], u4[:, h * 64 : (h + 1) * 64],
                                axis=mybir.AxisListType.X, op=OP.min,
                            )
                # combine pairs, clamp at 0, sqrt into roots
                min4 = work.tile([128, BPC], FP32, tag="min4", name="min4")
                nc.vector.tensor_reduce(
                    min4[:, :], mins2[:, :, :], axis=mybir.AxisListType.X, op=OP.min
                )
                nc.vector.tensor_scalar(
                    min4[:, :], min4[:, :], 0.0, None, op0=OP.max
                )
                nc.scalar.activation(
                    roots[:, nch * BPC : (nch + 1) * BPC], min4[:, :], AF.Sqrt
                )

            # ---- final: sum over the 64 roots columns ----
            acc = persist.tile([128, 1], FP32, tag="acc", name="acc")
            nc.vector.tensor_reduce(
                acc[:, :], roots[:, :], axis=mybir.AxisListType.X, op=OP.add
            )
            nc.sync.dma_start(out=out_ext[:, :], in_=acc[:, :])
            wout = persist.tile([128, 1], FP32, tag="wout", name="wout")
            nc.scalar.copy(wout[:, :], warmps[:, 0:1])
            nc.sync.dma_start(out=warm_ext[:, :], in_=wout[:, :])

    nc.compile()
    return nc


_NC_CACHE = None


def _get_nc():
    global _NC_CACHE
    if _NC_CACHE is None:
        _NC_CACHE = build_kernel()
    return _NC_CACHE


def make_in_maps(pred_R, pred_t, gt_R, gt_t, model_points):
    pointsT = np.ascontiguousarray(model_points.T.astype(np.float32))  # [3, N]
    in_maps = []
    for core in range(NCORES):
        Rp = np.zeros((3, 128), np.float32)
        Rg = np.zeros((3, 128), np.float32)
        biasp = np.zeros((128, 1), np.float32)
        biasg = np.zeros((128, 1), np.float32)
        onesp = np.zeros((128, 128), np.float32)
        onesg = np.zeros((128, 128), np.float32)
        for b in range(BPC):
            gb = core * BPC + b
            base = 32 * b
            Rp[:, base : base + 3] = pred_R[gb].T  # Rp[d, base+k] = pred_R[gb,k,d]
            Rg[:, base : base + 3] = gt_R[gb].T
            biasp[base : base + 3, 0] = -2.0 * pred_t[gb]
            biasg[base : base + 3, 0] = gt_t[gb]
            biasp[base + 4, 0] = 1.0  # pred ones row
            biasg[base + 3, 0] = 1.0  # gt ones row
            # pred pn2 at base+3 (0.25 * sum a^2, a = -2p); gt gn2 at base+4
            onesp[base : base + 3, base + 3] = 0.25
            onesg[base : base + 3, base + 4] = 1.0
        in_maps.append(
            {
                "pointsT": pointsT,
                "Rp": Rp,
                "Rg": Rg,
                "biasp": biasp,
                "biasg": biasg,
                "onesp": onesp,
                "onesg": onesg,
            }
        )
    return in_maps


def kernel(pred_R, pred_t, gt_R, gt_t, model_points):
    pred_R = np.asarray(pred_R, np.float32)
    pred_t = np.asarray(pred_t, np.float32)
    gt_R = np.asarray(gt_R, np.float32)
    gt_t = np.asarray(gt_t, np.float32)
    model_points = np.asarray(model_points, np.float32)

    nc = _get_nc()
    in_maps = make_in_maps(pred_R, pred_t, gt_R, gt_t, model_points)
    res = run_bass_kernel_spmd(nc, in_maps, core_ids=list(range(NCORES)))
    total = np.float64(0.0)
    for r in res.results:
        total += np.asarray(r["out"], np.float64).sum()
    return np.float32(total / (B * N))


# revision 17
# speedup vs baseline: 1.6387x; 1.6387x over previous
"""ADDS loss kernel for Trainium2, SPMD over 8 NeuronCores.

Problem: pred = model_points @ pred_R^T + pred_t (per batch), gt likewise;
d2[b,n,m] = ||pred[b,n] - gt[b,m]||^2; out = mean_{b,n} sqrt(max(min_m d2, 0)).

Sharding: data-parallel over batch B=32 -> 4 batches per core, one 5-row
operand group per batch at partition base 32*b:
  pred_stuff rows = [-2*p_x, -2*p_y, -2*p_z, pn2, 1]
  gt_stuff   rows = [g_x, g_y, g_z, 1, gn2]
so a K=5 matmul yields d2[n, m] = -2 p.g + pn2[n] + gn2[m] directly in PSUM.

Reduction: per (n_chunk, batch) the 4 m-chunk PSUM tiles are consumed by
ACT (bf16 convert of 2 tiles to SBUF) + DVE (2 chained tensor_tensor_scan
running-mins over (PSUM tile, SBUF tile) pairs) -> [128,1] min with no
reduce tail. Then clamp, sqrt, sum. Host sums the 8x[128,1] partials.
"""

import numpy as np

import concourse.bass as bass
import concourse.bacc as bacc_mod
import concourse.mybir as mybir
from concourse.tile import TileContext
from concourse.bass_utils import run_bass_kernel_spmd

B = 32
N = 2048
NCORES = 8
BPC = B // NCORES  # batches per core = 4
FP32 = mybir.dt.float32
BF16 = mybir.dt.bfloat16
AF = mybir.ActivationFunctionType
OP = mybir.AluOpType

# float32r streams at 1 col/cycle for moving dims >= 256 (vs 1/4 for fp32)
USE_F32R = True
BIG_INIT = 1.0e30
NFILL = 3


def _mm_dt(ap):
    return ap.bitcast(mybir.dt.float32r) if USE_F32R else ap


def build_kernel():
    nc = bacc_mod.Bacc()

    pointsT_ext = nc.declare_dram_parameter("pointsT", [3, N], FP32, isOutput=False)
    Rp_ext = nc.declare_dram_parameter("Rp", [3, 128], FP32, isOutput=False)
    Rg_ext = nc.declare_dram_parameter("Rg", [3, 128], FP32, isOutput=False)
    biasp_ext = nc.declare_dram_parameter("biasp", [128, 1], FP32, isOutput=False)
    biasg_ext = nc.declare_dram_parameter("biasg", [128, 1], FP32, isOutput=False)
    onesp_ext = nc.declare_dram_parameter("onesp", [128, 128], FP32, isOutput=False)
    onesg_ext = nc.declare_dram_parameter("onesg", [128, 128], FP32, isOutput=False)
    out_ext = nc.declare_dram_parameter("out", [128, 1], FP32, isOutput=True)

    with TileContext(nc) as tc:
        with (
            tc.tile_pool(name="persist", bufs=1) as persist,
            tc.tile_pool(name="work", bufs=2) as work,
            tc.tile_pool(name="sbf", bufs=3) as sbf,
            tc.tile_pool(name="ps", bufs=4, space="PSUM") as ps,
        ):
            # ---- load inputs ----
            def load(ext, shape, nm):
                t = persist.tile(shape, FP32, tag=nm, name=nm)
                nc.sync.dma_start(out=t[:, :], in_=ext[:, :])
                return t

            pointsT = load(pointsT_ext, [3, N], "pointsT_sb")
            Rsb = {}
            biassb = {}
            onessb = {}
            for side, (R_ext, b_ext, o_ext) in (
                ("p", (Rp_ext, biasp_ext, onesp_ext)),
                ("g", (Rg_ext, biasg_ext, onesg_ext)),
            ):
                Rsb[side] = load(R_ext, [3, 128], f"R{side}_sb")
                biassb[side] = load(b_ext, [128, 1], f"bias{side}_sb")
                onessb[side] = load(o_ext, [128, 128], f"ones{side}_sb")

            # ---- Phase A: build stuff_p / stuff_g (all f32r) ----
            F32R = mybir.dt.float32r
            pointsT_r = persist.tile([3, N], F32R, tag="pointsT_r", name="pointsT_r")
            nc.scalar.copy(pointsT_r[:, :], pointsT[:, :])
            stuff = {}
            for side, scale in (("p", -2.0), ("g", 1.0)):
                R_r = persist.tile([3, 128], F32R, tag=f"R{side}_r", name=f"R{side}_r")
                nc.scalar.copy(R_r[:, :], Rsb[side][:, :])
                ones_r = persist.tile(
                    [128, 128], F32R, tag=f"ones{side}_r", name=f"ones{side}_r"
                )
                nc.scalar.copy(ones_r[:, :], onessb[side][:, :])
                stp = persist.tile([128, N], F32R, tag=f"stp{side}", name=f"stp{side}_sb")
                for c in range(N // 512):
                    T = ps.tile([128, 512], FP32, tag="psb", name="psb")
                    nc.tensor.matmul(
                        T[:, :],
                        R_r[:, :],
                        pointsT_r[:, c * 512 : (c + 1) * 512],
                        start=True,
                        stop=True,
                    )
                    # stuff = scale*transform + bias (rounded to f32r)
                    nc.scalar.activation(
                        stp[:, c * 512 : (c + 1) * 512], T[:, :], AF.Identity,
                        bias=biassb[side][:, :], scale=scale,
                    )
                # square on ACT (f32r out feeds the norms matmul)
                sq = work.tile([128, N], F32R, tag="sq", name="sq")
                nc.scalar.activation(sq[:, :], stp[:, :], AF.Square)
                # norms matmul: N_ps[row, n] = sum_p ones[p, row] * sq[p, n]
                for c in range(N // 512):
                    N_ps = ps.tile([128, 512], FP32, tag="psb", name="psb")
                    nc.tensor.matmul(
                        N_ps[:, :],
                        ones_r[:, :],
                        sq[:, c * 512 : (c + 1) * 512],
                        start=True,
                        stop=True,
                    )
                    # fold norm rows into stuff (other rows of N_ps are 0)
                    nc.vector.tensor_tensor(
                        stp[:, c * 512 : (c + 1) * 512],
                        stp[:, c * 512 : (c + 1) * 512],
                        N_ps[:, :],
                        op=OP.add,
                    )
                stuff[side] = stp

            # ---- Phase B: main loop ----
            # Per (nch, b) group: two [128,1024] PSUM tiles (2 banks each,
            # 2 f32r matmuls per tile). Groups alternate consumers:
            #  - DVE-direct: reduce_min over each 1024-wide tile.
            #  - ACT+tree: ACT converts each tile half to bf16 SBUF, DVE runs
            #    a 2x TT-min tree (1024->512->256->128) + split final reduce.
            # Every group emits 2 partial mins into mins2[:, b, 0:2].
            roots = persist.tile([128, 16 * BPC], FP32, tag="roots", name="roots")
            for nch in range(16):
                mins2 = work.tile([128, BPC, 2], FP32, tag="mins2", name="mins2")
                for b in range(BPC):
                    g = nch * BPC + b
                    lhs = stuff["p"][32 * b : 32 * b + 5, nch * 128 : (nch + 1) * 128]
                    halves = []
                    for h in range(2):
                        P = ps.tile([128, 1024], FP32, tag="psb", name="psb")
                        halves.append(P)
                        for mc in range(2):
                            m0 = (2 * h + mc) * 512
                            nc.tensor.matmul(
                                P[:, mc * 512 : (mc + 1) * 512],
                                lhs,
                                stuff["g"][32 * b : 32 * b + 5, m0 : m0 + 512],
                                start=True,
                                stop=True,
                                tile_position=(32 * b, 0),
                            )
                    if g % 7 == 6:
                        # DVE-direct group
                        for h in range(2):
                            nc.vector.tensor_reduce(
                                mins2[:, b, h : h + 1], halves[h][:, :],
                                axis=mybir.AxisListType.X, op=OP.min,
                            )
                    else:
                        S = sbf.tile([128, 2048], BF16, tag="S", name="S")
                        for h in range(2):
                            nc.scalar.copy(
                                S[:, h * 1024 : (h + 1) * 1024], halves[h][:, :]
                            )
                        u1 = sbf.tile([128, 1024], BF16, tag="u1", name="u1")
                        nc.vector.tensor_tensor(
                            u1[:, :], S[:, 0:1024], S[:, 1024:2048], op=OP.min
                        )
                        u2 = sbf.tile([128, 512], BF16, tag="u2", name="u2")
                        nc.vector.tensor_tensor(
                            u2[:, :], u1[:, 0:512], u1[:, 512:1024], op=OP.min
                        )
                        u3 = sbf.tile([128, 256], BF16, tag="u3", name="u3")
                        nc.vector.tensor_tensor(
                            u3[:, :], u2[:, 0:256], u2[:, 256:512], op=OP.min
                        )
                        u4 = sbf.tile([128, 128], BF16, tag="u4", name="u4")
                        nc.vector.tensor_tensor(
                            u4[:, :], u3[:, 0:128], u3[:, 128:256], op=OP.min
                        )
                        for h in range(2):
                            nc.vector.tensor_reduce(
                                mins2[:, b, h : h + 1], u4[:, h * 64 : (h + 1) * 64],
                                axis=mybir.AxisListType.X, op=OP.min,
                            )
                # combine pairs, clamp at 0, sqrt into roots
                min4 = work.tile([128, BPC], FP32, tag="min4", name="min4")
                nc.vector.tensor_reduce(
                    min4[:, :], mins2[:, :, :], axis=mybir.AxisListType.X, op=OP.min
                )
                nc.vector.tensor_scalar(
                    min4[:, :], min4[:, :], 0.0, None, op0=OP.max
                )
                nc.scalar.activation(
                    roots[:, nch * BPC : (nch + 1) * BPC], min4[:, :], AF.Sqrt
                )

            # ---- final: sum over the 64 roots columns ----
            acc = persist.tile([128, 1], FP32, tag="acc", name="acc")
            nc.vector.tensor_reduce(
                acc[:, :], roots[:, :], axis=mybir.AxisListType.X, op=OP.add
            )
            nc.sync.dma_start(out=out_ext[:, :], in_=acc[:, :])

    nc.compile()
    return nc


_NC_CACHE = None


def _get_nc():
    global _NC_CACHE
    if _NC_CACHE is None:
        _NC_CACHE = build_kernel()
    return _NC_CACHE


def make_in_maps(pred_R, pred_t, gt_R, gt_t, model_points):
    pointsT = np.ascontiguousarray(model_points.T.astype(np.float32))  # [3, N]
    in_maps = []
    for core in range(NCORES):
        Rp = np.zeros((3, 128), np.float32)
        Rg = np.zeros((3, 128), np.float32)
        biasp = np.zeros((128, 1), np.float32)
        biasg = np.zeros((128, 1), np.float32)
        onesp = np.zeros((128, 128), np.float32)
        onesg = np.zeros((128, 128), np.float32)
        for b in range(BPC):
            gb = core * BPC + b
            base = 32 * b
            Rp[:, base : base + 3] = pred_R[gb].T  # Rp[d, base+k] = pred_R[gb,k,d]
            Rg[:, base : base + 3] = gt_R[gb].T
            biasp[base : base + 3, 0] = -2.0 * pred_t[gb]
            biasg[base : base + 3, 0] = gt_t[gb]
            biasp[base + 4, 0] = 1.0  # pred ones row
            biasg[base + 3, 0] = 1.0  # gt ones row
            # pred pn2 at base+3 (0.25 * sum a^2, a = -2p); gt gn2 at base+4
            onesp[base : base + 3, base + 3] = 0.25
            onesg[base : base + 3, base + 4] = 1.0
        in_maps.append(
            {
                "pointsT": pointsT,
                "Rp": Rp,
                "Rg": Rg,
                "biasp": biasp,
                "biasg": biasg,
                "onesp": onesp,
                "onesg": onesg,
            }
        )
    return in_maps


def kernel(pred_R, pred_t, gt_R, gt_t, model_points):
    pred_R = np.asarray(pred_R, np.float32)
    pred_t = np.asarray(pred_t, np.float32)
    gt_R = np.asarray(gt_R, np.float32)
    gt_t = np.asarray(gt_t, np.float32)
    model_points = np.asarray(model_points, np.float32)

    nc = _get_nc()
    in_maps = make_in_maps(pred_R, pred_t, gt_R, gt_t, model_points)
    res = run_bass_kernel_spmd(nc, in_maps, core_ids=list(range(NCORES)))
    total = np.float64(0.0)
    for r in res.results:
        total += np.asarray(r["out"], np.float64).sum()
    return np.float32(total / (B * N))
